# revision 3
# baseline (speedup 1.0000x reference)
"""AdaptiveGraphLearner distributed Trainium2 kernel (8 NeuronCores), v4.

reference:  sim = (x @ x.T)/0.1;  adj = sim * rowwise_top32_mask(sim)
            out = (adj + adj.T)/2

Same math as v3 (row-sharded; per-row e32/e33 threshold midpoints; 4KB
AllGather of column thresholds instead of a 32MB adj transpose), with the
schedule rebuilt around the measured bottlenecks:

- ONE AllGather (v3 had two serialized ~21us collectives) into a single
  shared buffer; cb columns land in natural order so the broadcast is 8
  plain DMAs.
- Bridge staging: while the AllGather + cb broadcast fly, phase 3 starts
  anyway -- the first S halves matmul into PSUM and ACT stages the raw
  fp32 into SBUF (hf32), freeing PSUM so the PE keeps streaming; their
  compares run from hf32 once cb arrives.
- Phase 3 engine rebalance: per [128,1024] tile the ops are
    h5 = 5*psum (ACT, bf16 values),
    row mask    (variant A: ACT saturated sigmoid | variant B: DVE
                 scalar_tensor_tensor (psum > tmid) + cc, fused add),
    cc = psum > cb (DVE),
    m = mr + cc (DVE bf16, A-variant only),
    ob = h5 * m (mostly GpSimd, some DVE).
  Mix chosen so ACT/DVE/Pool all sit near the same per-tile time.
- Output is bf16; host upcasts to fp32.
"""
import sys
sys.path.insert(0, '/opt/trn_rl_repo')
import numpy as np
import concourse.bass as bass
import concourse.bacc as bacc
import concourse.mybir as mybir
import concourse.tile as tile
from concourse.bass_utils import run_bass_kernel_spmd

N, DIM, K = 8192, 256, 32
TEMP = 0.1
SCALE = 0.5 / TEMP
NCORES = 8
RPC = N // NCORES          # 1024 rows per core
NB = RPC // 128            # 8 row-blocks of 128
NEG = -1e30
SIGBIG = 1.0e6
NSTAGE = 2                 # halves staged through SBUF during the bridge

f32 = mybir.dt.float32
f32r = mybir.dt.float32r
bf16 = mybir.dt.bfloat16
COPY = mybir.ActivationFunctionType.Copy
SIG = mybir.ActivationFunctionType.Sigmoid
GT = mybir.AluOpType.is_gt
ADD = mybir.AluOpType.add
MUL = mybir.AluOpType.mult


def build_nc():
    nc = bacc.Bacc(None, target_bir_lowering=False, num_devices=NCORES)
    xT = nc.declare_dram_parameter("xT", [DIM, N], f32r, isOutput=False)
    xgT = nc.declare_dram_parameter("xgT", [DIM, RPC], f32r, isOutput=False)
    out = nc.declare_dram_parameter("out", [RPC, N], bf16, isOutput=True)

    with tile.TileContext(nc) as tc:
        with tc.tile_pool(name="dram", bufs=1, space="DRAM") as dram:
            t_loc = dram.tile([RPC], f32)
            t_all = dram.tile([NCORES * RPC], f32, addr_space="Shared")

            with tc.tile_pool(name="keep", bufs=1) as keep:
                tmids = keep.tile([128, NB], f32, name="tmids", tag="tmd")
                sgbias = keep.tile([128, NB], f32, name="sgbias", tag="sgb")
                xr0 = keep.tile([128, N], f32r, name="xr0", tag="xr0")
                xr1 = keep.tile([128, N], f32r, name="xr1", tag="xr1")
                xg0 = keep.tile([128, RPC], f32r, name="xg0", tag="xg0")
                xg1 = keep.tile([128, RPC], f32r, name="xg1", tag="xg1")
                cb = keep.tile([128, N], f32, name="cb", tag="cb")

                # PE warmup: dummy matmuls to start the p-state ramp
                with tc.tile_pool(name="warm", bufs=1) as warm, \
                     tc.tile_pool(name="wps", bufs=1, space="PSUM") as wps:
                    wsf = warm.tile([128, 512], f32, name="wsf", tag="wf")
                    wsrc = warm.tile([128, 512], f32r, name="wsrc", tag="ws")
                    wp = wps.tile([128, 512], f32, name="wp", tag="wp")
                    nc.vector.memset(wsf[:], 0.0)
                    nc.scalar.activation(wsrc[:], wsf[:], COPY)
                    for _ in range(10):
                        nc.tensor.matmul(wp[:], wsrc[:, 0:128], wsrc[:],
                                         start=True, stop=True)

                # chunked input loads (first matmuls start early)
                nc.sync.dma_start(xg0[:], xgT[0:128, :])
                nc.sync.dma_start(xg1[:], xgT[128:256, :])
                bounds = [0, 256, 512, 1024, 2048, 3072, 4096, 6144, 8192]
                for c in range(len(bounds) - 1):
                    c0, c1 = bounds[c], bounds[c + 1]
                    nc.sync.dma_start(xr0[:, c0:c1], xT[0:128, c0:c1])
                    nc.sync.dma_start(xr1[:, c0:c1], xT[128:256, c0:c1])

                def mega_matmuls(ps_pool, rb, half, tag):
                    """Four [128,1024] psum tiles (= one half-block of 4096
                    cols); each stationary serves 8 consecutive MMs."""
                    r0, r1 = rb * 128, (rb + 1) * 128
                    base = half * 4096
                    mg = [ps_pool.tile([128, 1024], f32, name="mg", tag=tag)
                          for _ in range(4)]
                    for g in range(4):
                        for t in range(2):
                            c0 = base + g * 1024 + t * 512
                            nc.tensor.matmul(mg[g][:, t * 512:(t + 1) * 512],
                                             xg0[:, r0:r1],
                                             xr0[:, c0:c0 + 512],
                                             start=True, stop=False)
                    for g in range(4):
                        for t in range(2):
                            c0 = base + g * 1024 + t * 512
                            nc.tensor.matmul(mg[g][:, t * 512:(t + 1) * 512],
                                             xg1[:, r0:r1],
                                             xr1[:, c0:c0 + 512],
                                             start=False, stop=True)
                    return mg

                # ---------------- Phase 1: thresholds ----------------
                with tc.tile_pool(name="ps1", bufs=4, space="PSUM") as ps1, \
                     tc.tile_pool(name="thr", bufs=1) as thr:
                    cand = thr.tile([128, 256], f32, name="cand", tag="cand")
                    m8x = thr.tile([128, 17], f32, name="m8x", tag="m8x")
                    m8a, m8b, tmid = m8x[:, 0:8], m8x[:, 8:16], m8x[:, 16:17]
                    for rb in range(NB):
                        for half in range(2):
                            mg = mega_matmuls(ps1, rb, half, "p")
                            for g in range(4):
                                mi = half * 4 + g
                                for ch in range(4):
                                    o = mi * 32 + ch * 8
                                    nc.vector.max(
                                        out=cand[:, o:o + 8],
                                        in_=mg[g][:, ch * 256:(ch + 1) * 256])
                        for r in range(4):
                            nc.vector.max(out=m8a, in_=cand[:])
                            nc.vector.match_replace(out=cand[:],
                                                    in_to_replace=m8a,
                                                    in_values=cand[:],
                                                    imm_value=NEG)
                        nc.vector.max(out=m8b, in_=cand[:])
                        nc.vector.tensor_add(tmid, m8a[:, 7:8], m8b[:, 0:1])
                        nc.vector.tensor_scalar_mul(tmid, tmid, 0.5)
                        nc.vector.tensor_copy(tmids[:, rb:rb + 1], tmid)
                        nc.vector.tensor_scalar_mul(
                            sgbias[:, rb:rb + 1], tmid, -float(SIGBIG))
                        nc.sync.dma_start(
                            t_loc[rb * 128:(rb + 1) * 128], tmid)

                # ---------------- AllGather (single) ----------------
                nc.gpsimd.collective_compute(
                    "AllGather", mybir.AluOpType.bypass,
                    replica_groups=[list(range(NCORES))],
                    ins=[t_loc.opt()], outs=[t_all.opt()])

                # cb[:, j] = threshold of global row j, broadcast down the
                # 128 partitions. t_all is already in natural row order.
                for c in range(NCORES):
                    nc.sync.dma_start(
                        cb[:, c * RPC:(c + 1) * RPC],
                        t_all.tensor.reshape([1, NCORES * RPC])
                        .ap()[:, c * RPC:(c + 1) * RPC]
                        .to_broadcast((128, RPC)))

                # ---------------- Phase 3: recompute + mask ----------------
                # Halves 0..NSTAGE-1 are staged: ACT copies raw psum to SBUF
                # fp32 (hf32) so PSUM frees and the PE streams on while the
                # AllGather/broadcast are in flight; their compares read
                # hf32 later. Steady halves compare straight from PSUM.
                with tc.tile_pool(name="ps3", bufs=4, space="PSUM") as ps3, \
                     tc.tile_pool(name="stg", bufs=4 * NSTAGE) as stg, \
                     tc.tile_pool(name="smk", bufs=4 * NSTAGE) as smk, \
                     tc.tile_pool(name="hs", bufs=6) as hsp, \
                     tc.tile_pool(name="mk", bufs=4) as mk, \
                     tc.tile_pool(name="ob", bufs=2) as obp:

                    staged = []   # (rb, half, [hf32 x4], [h5 x4], [mr x4])
                    # ---- bridge: matmul + stage the first NSTAGE halves
                    for s in range(NSTAGE):
                        rb, half = s // 2, s % 2
                        mg = mega_matmuls(ps3, rb, half, "q")
                        hfs, h5s, mrs = [], [], []
                        for g in range(4):
                            hf = stg.tile([128, 1024], f32, name="hf",
                                          tag="hf")
                            nc.scalar.activation(hf[:], mg[g][:], COPY)
                            h5 = hsp.tile([128, 1024], bf16, name="h5",
                                          tag="h5")
                            nc.scalar.activation(h5[:], mg[g][:], COPY,
                                                 scale=float(SCALE))
                            mr = smk.tile([128, 1024], bf16, name="smr",
                                          tag="smr")
                            if g < 2:
                                # ACT saturated sigmoid (exact 0/1)
                                nc.scalar.activation(
                                    mr[:], hf[:], SIG, scale=float(SIGBIG),
                                    bias=sgbias[:, rb:rb + 1])
                            else:
                                # DVE row compare (cb-independent)
                                nc.vector.tensor_scalar(
                                    out=mr[:], in0=hf[:],
                                    scalar1=tmids[:, rb:rb + 1],
                                    scalar2=None, op0=GT)
                            hfs.append(hf)
                            h5s.append(h5)
                            mrs.append(mr)
                        staged.append((rb, half, hfs, h5s, mrs))

                    # ---- staged halves: compares once cb is ready
                    for rb, half, hfs, h5s, mrs in staged:
                        ob = obp.tile([128, 4096], bf16, name="ob", tag="ob")
                        for g in range(4):
                            mi = half * 4 + g
                            c0 = mi * 1024
                            cc = mk.tile([128, 1024], bf16, name="cc",
                                         tag="cc")
                            nc.vector.tensor_tensor(
                                out=cc[:], in0=hfs[g][:],
                                in1=cb[:, c0:c0 + 1024], op=GT)
                            m = mk.tile([128, 1024], bf16, name="m", tag="m")
                            nc.vector.tensor_tensor(
                                out=m[:], in0=mrs[g][:], in1=cc[:], op=ADD)
                            if g == 3:
                                nc.vector.tensor_tensor(
                                    out=ob[:, g * 1024:(g + 1) * 1024],
                                    in0=h5s[g][:], in1=m[:], op=MUL)
                            else:
                                nc.gpsimd.tensor_tensor(
                                    out=ob[:, g * 1024:(g + 1) * 1024],
                                    in0=h5s[g][:], in1=m[:], op=MUL)
                        r0 = rb * 128
                        g0 = half * 4096
                        nc.sync.dma_start(out[r0:r0 + 128, g0:g0 + 4096],
                                          ob[:])

                    # ---- steady halves
                    for s in range(NSTAGE, 2 * NB):
                        rb, half = s // 2, s % 2
                        r0 = rb * 128
                        mg = mega_matmuls(ps3, rb, half, "q")
                        ob = obp.tile([128, 4096], bf16, name="ob", tag="ob")
                        for g in range(4):
                            mi = half * 4 + g
                            c0 = mi * 1024
                            h5 = hsp.tile([128, 1024], bf16, name="h5",
                                          tag="h5")
                            nc.scalar.activation(h5[:], mg[g][:], COPY,
                                                 scale=float(SCALE))
                            cc = mk.tile([128, 1024], bf16, name="cc",
                                         tag="cc")
                            nc.vector.tensor_tensor(
                                out=cc[:], in0=mg[g][:],
                                in1=cb[:, c0:c0 + 1024], op=GT)
                            m = mk.tile([128, 1024], bf16, name="m", tag="m")
                            if g == 0:
                                # variant B: fused row-compare + add on DVE
                                nc.vector.scalar_tensor_tensor(
                                    out=m[:], in0=mg[g][:],
                                    scalar=tmids[:, rb:rb + 1],
                                    in1=cc[:], op0=GT, op1=ADD)
                            else:
                                # variant A: ACT sigmoid row mask + DVE add
                                ia = mk.tile([128, 1024], bf16, name="ia",
                                             tag="ia")
                                nc.scalar.activation(
                                    ia[:], mg[g][:], SIG,
                                    scale=float(SIGBIG),
                                    bias=sgbias[:, rb:rb + 1])
                                nc.vector.tensor_tensor(
                                    out=m[:], in0=ia[:], in1=cc[:], op=ADD)
                            if g == 3:
                                nc.vector.tensor_tensor(
                                    out=ob[:, g * 1024:(g + 1) * 1024],
                                    in0=h5[:], in1=m[:], op=MUL)
                            else:
                                nc.gpsimd.tensor_tensor(
                                    out=ob[:, g * 1024:(g + 1) * 1024],
                                    in0=h5[:], in1=m[:], op=MUL)
                        g0 = half * 4096
                        nc.sync.dma_start(out[r0:r0 + 128, g0:g0 + 4096],
                                          ob[:])

    nc.compile()
    return nc


_nc_cache = None


def get_nc():
    global _nc_cache
    if _nc_cache is None:
        _nc_cache = build_nc()
    return _nc_cache


def kernel_with_result(x, trace: bool = False):
    x = np.ascontiguousarray(np.asarray(x), dtype=np.float32)
    assert x.shape == (N, DIM)
    nc = get_nc()
    xT = np.ascontiguousarray(x.T)
    in_maps = []
    for i in range(NCORES):
        xg = np.ascontiguousarray(x[i * RPC:(i + 1) * RPC, :].T)
        in_maps.append({"xT": xT, "xgT": xg})
    res = run_bass_kernel_spmd(nc, in_maps, core_ids=list(range(NCORES)),
                               trace=trace)
    outp = np.concatenate(
        [np.asarray(res.results[i]["out"]).astype(np.float32)
         for i in range(NCORES)], axis=0)
    return outp, res


def kernel(x) -> np.ndarray:
    outp, _res = kernel_with_result(x)
    return outp


# revision 4
# speedup vs baseline: 1.3148x; 1.3148x over previous
"""AdaptiveGraphLearner distributed Trainium2 kernel (8 NeuronCores), v5.

reference:  sim = (x @ x.T)/0.1;  adj = sim * rowwise_top32_mask(sim)
            out = (adj + adj.T)/2

Row-sharded across 8 cores; per-row e32/e33 threshold midpoints; one 4KB
AllGather of column thresholds instead of a 32MB adj transpose. Schedule:

- PSUM double buffering: work is cut into 2048-col units (2 x [128,1024]
  fp32 psum tiles = 4 banks), pool bufs=4 keeps TWO units in flight so the
  PE streams while the DVE consumes the previous unit (v3/v4 had one
  8-bank half in flight, serializing PE vs DVE every 4096 cols).
- Phase 1 per row-block: DVE max8 top-8 per 256-col chunk -> 256
  candidates, 5 rounds of max8+match_replace -> e32/e33. The tiny
  per-row-block threshold math (add/scale) runs on GpSimd, keeping DVE on
  scans only.
- ONE AllGather; cb columns land in natural order; 8 broadcast DMAs.
- Bridge staging: while the AllGather+broadcast fly, the first NSTAGE
  phase-3 units matmul into PSUM and ACT stages raw fp32 to SBUF (hf32),
  freeing PSUM; their compares run from hf32 once cb arrives.
- Phase 3 per [128,1024] tile: ACT h5 = 5*psum (bf16 values) + ACT
  saturated-sigmoid row mask; DVE column compare vs cb + bf16 mask add;
  final h5*m bf16 multiply alternates DVE / GpSimd.
- Output bf16; host upcasts.
"""
import sys
sys.path.insert(0, '/opt/trn_rl_repo')
import numpy as np
import concourse.bass as bass
import concourse.bacc as bacc
import concourse.mybir as mybir
import concourse.tile as tile
from concourse.bass_utils import run_bass_kernel_spmd

N, DIM, K = 8192, 256, 32
TEMP = 0.1
SCALE = 0.5 / TEMP
NCORES = 8
RPC = N // NCORES          # 1024 rows per core
NB = RPC // 128            # 8 row-blocks of 128
UPB = 4                    # 2048-col units per row-block
NEG = -1e30
SIGBIG = 1.0e6
NSTAGE = 3                 # units staged through SBUF during the bridge

f32 = mybir.dt.float32
f32r = mybir.dt.float32r
bf16 = mybir.dt.bfloat16
COPY = mybir.ActivationFunctionType.Copy
SIG = mybir.ActivationFunctionType.Sigmoid
GT = mybir.AluOpType.is_gt
ADD = mybir.AluOpType.add
MUL = mybir.AluOpType.mult


def build_nc():
    nc = bacc.Bacc(None, target_bir_lowering=False, num_devices=NCORES)
    xT = nc.declare_dram_parameter("xT", [DIM, N], f32r, isOutput=False)
    xgT = nc.declare_dram_parameter("xgT", [DIM, RPC], f32r, isOutput=False)
    out = nc.declare_dram_parameter("out", [RPC, N], bf16, isOutput=True)

    with tile.TileContext(nc) as tc:
        with tc.tile_pool(name="dram", bufs=1, space="DRAM") as dram:
            t_loc = dram.tile([RPC], f32)
            t_all = dram.tile([NCORES * RPC], f32, addr_space="Shared")

            with tc.tile_pool(name="keep", bufs=1) as keep:
                sgbias = keep.tile([128, NB], f32, name="sgbias", tag="sgb")
                xr0 = keep.tile([128, N], f32r, name="xr0", tag="xr0")
                xr1 = keep.tile([128, N], f32r, name="xr1", tag="xr1")
                xg0 = keep.tile([128, RPC], f32r, name="xg0", tag="xg0")
                xg1 = keep.tile([128, RPC], f32r, name="xg1", tag="xg1")
                cb = keep.tile([128, N], f32, name="cb", tag="cb")

                # PE warmup: dummy matmuls to start the p-state ramp
                with tc.tile_pool(name="warm", bufs=1) as warm, \
                     tc.tile_pool(name="wps", bufs=1, space="PSUM") as wps:
                    wsf = warm.tile([128, 512], f32, name="wsf", tag="wf")
                    wsrc = warm.tile([128, 512], f32r, name="wsrc", tag="ws")
                    wp = wps.tile([128, 512], f32, name="wp", tag="wp")
                    nc.vector.memset(wsf[:], 0.0)
                    nc.scalar.activation(wsrc[:], wsf[:], COPY)
                    for _ in range(10):
                        nc.tensor.matmul(wp[:], wsrc[:, 0:128], wsrc[:],
                                         start=True, stop=True)

                # chunked input loads (first matmuls start early)
                nc.sync.dma_start(xg0[:], xgT[0:128, :])
                nc.sync.dma_start(xg1[:], xgT[128:256, :])
                bounds = [0, 256, 512, 1024, 2048, 3072, 4096, 6144, 8192]
                for c in range(len(bounds) - 1):
                    c0, c1 = bounds[c], bounds[c + 1]
                    nc.sync.dma_start(xr0[:, c0:c1], xT[0:128, c0:c1])
                    nc.sync.dma_start(xr1[:, c0:c1], xT[128:256, c0:c1])

                def unit_matmuls(ps_pool, rb, u, tag):
                    """One 2048-col unit: 2 x [128,1024] psum tiles."""
                    r0, r1 = rb * 128, (rb + 1) * 128
                    base = u * 2048
                    mg = [ps_pool.tile([128, 1024], f32, name="mg", tag=tag)
                          for _ in range(2)]
                    for t in range(2):
                        for s in range(2):
                            c0 = base + t * 1024 + s * 512
                            nc.tensor.matmul(mg[t][:, s * 512:(s + 1) * 512],
                                             xg0[:, r0:r1],
                                             xr0[:, c0:c0 + 512],
                                             start=True, stop=False)
                    for t in range(2):
                        for s in range(2):
                            c0 = base + t * 1024 + s * 512
                            nc.tensor.matmul(mg[t][:, s * 512:(s + 1) * 512],
                                             xg1[:, r0:r1],
                                             xr1[:, c0:c0 + 512],
                                             start=False, stop=True)
                    return mg

                # ---------------- Phase 1: thresholds ----------------
                with tc.tile_pool(name="ps1", bufs=4, space="PSUM") as ps1, \
                     tc.tile_pool(name="thr", bufs=1) as thr, \
                     tc.tile_pool(name="m8p", bufs=2) as m8p:
                    cand = thr.tile([128, 256], f32, name="cand", tag="cand")
                    for rb in range(NB):
                        for u in range(UPB):
                            mg = unit_matmuls(ps1, rb, u, "p")
                            for t in range(2):
                                for ch in range(4):
                                    o = u * 64 + t * 32 + ch * 8
                                    nc.vector.max(
                                        out=cand[:, o:o + 8],
                                        in_=mg[t][:, ch * 256:(ch + 1) * 256])
                        m8x = m8p.tile([128, 17], f32, name="m8x", tag="m8x")
                        m8a, m8b = m8x[:, 0:8], m8x[:, 8:16]
                        tmid = m8x[:, 16:17]
                        for r in range(4):
                            nc.vector.max(out=m8a, in_=cand[:])
                            nc.vector.match_replace(out=cand[:],
                                                    in_to_replace=m8a,
                                                    in_values=cand[:],
                                                    imm_value=NEG)
                        nc.vector.max(out=m8b, in_=cand[:])
                        # tiny threshold math on GpSimd (keep DVE scanning)
                        nc.gpsimd.tensor_add(tmid, m8a[:, 7:8], m8b[:, 0:1])
                        nc.gpsimd.tensor_scalar_mul(tmid, tmid, 0.5)
                        nc.gpsimd.tensor_scalar_mul(
                            sgbias[:, rb:rb + 1], tmid, -float(SIGBIG))
                        nc.sync.dma_start(
                            t_loc[rb * 128:(rb + 1) * 128], tmid)

                # ---------------- AllGather (single) ----------------
                nc.gpsimd.collective_compute(
                    "AllGather", mybir.AluOpType.bypass,
                    replica_groups=[list(range(NCORES))],
                    ins=[t_loc.opt()], outs=[t_all.opt()])

                # cb[:, j] = threshold of global row j, broadcast down the
                # 128 partitions. t_all is already in natural row order.
                for c in range(NCORES):
                    nc.sync.dma_start(
                        cb[:, c * RPC:(c + 1) * RPC],
                        t_all.tensor.reshape([1, NCORES * RPC])
                        .ap()[:, c * RPC:(c + 1) * RPC]
                        .to_broadcast((128, RPC)))

                # ---------------- Phase 3: recompute + mask ----------------
                with tc.tile_pool(name="ps3", bufs=4, space="PSUM") as ps3, \
                     tc.tile_pool(name="stg", bufs=2 * NSTAGE) as stg, \
                     tc.tile_pool(name="smk", bufs=2 * NSTAGE) as smk, \
                     tc.tile_pool(name="hs", bufs=6) as hsp, \
                     tc.tile_pool(name="mk", bufs=3) as mk, \
                     tc.tile_pool(name="ob", bufs=4) as obp:

                    def masked_tile(src, h5, mr, ob, gslot, mi):
                        """Column compare + mask add + value multiply for one
                        [128,1024] tile; mul alternates DVE/GpSimd."""
                        c0 = mi * 1024
                        cc = mk.tile([128, 1024], bf16, name="cc", tag="cc")
                        nc.vector.tensor_tensor(
                            out=cc[:], in0=src[:],
                            in1=cb[:, c0:c0 + 1024], op=GT)
                        m = mk.tile([128, 1024], bf16, name="m", tag="m")
                        nc.vector.tensor_tensor(
                            out=m[:], in0=mr[:], in1=cc[:], op=ADD)
                        o = ob[:, gslot * 1024:(gslot + 1) * 1024]
                        if mi % 2 == 1:
                            nc.vector.tensor_tensor(out=o, in0=h5[:],
                                                    in1=m[:], op=MUL)
                        else:
                            nc.gpsimd.tensor_tensor(out=o, in0=h5[:],
                                                    in1=m[:], op=MUL)

                    staged = []   # (rb, u, [h5 x2], [hf x2], [mr x2])
                    # ---- bridge: matmul + stage the first NSTAGE units
                    for s in range(NSTAGE):
                        rb, u = s // UPB, s % UPB
                        mg = unit_matmuls(ps3, rb, u, "q")
                        hfs, h5s, mrs = [], [], []
                        for t in range(2):
                            hf = stg.tile([128, 1024], f32, name="hf",
                                          tag="hf")
                            nc.scalar.activation(hf[:], mg[t][:], COPY)
                            h5 = hsp.tile([128, 1024], bf16, name="h5",
                                          tag="h5")
                            nc.scalar.activation(h5[:], mg[t][:], COPY,
                                                 scale=float(SCALE))
                            mr = smk.tile([128, 1024], bf16, name="smr",
                                          tag="smr")
                            nc.scalar.activation(
                                mr[:], hf[:], SIG, scale=float(SIGBIG),
                                bias=sgbias[:, rb:rb + 1])
                            hfs.append(hf)
                            h5s.append(h5)
                            mrs.append(mr)
                        staged.append((rb, u, hfs, h5s, mrs))

                    # ---- staged units: compares once cb is ready
                    for rb, u, hfs, h5s, mrs in staged:
                        ob = obp.tile([128, 2048], bf16, name="ob", tag="ob")
                        for t in range(2):
                            mi = u * 2 + t
                            masked_tile(hfs[t], h5s[t], mrs[t], ob, t, mi)
                        r0 = rb * 128
                        g0 = u * 2048
                        nc.sync.dma_start(out[r0:r0 + 128, g0:g0 + 2048],
                                          ob[:])

                    # ---- steady units
                    for s in range(NSTAGE, NB * UPB):
                        rb, u = s // UPB, s % UPB
                        r0 = rb * 128
                        mg = unit_matmuls(ps3, rb, u, "q")
                        ob = obp.tile([128, 2048], bf16, name="ob", tag="ob")
                        for t in range(2):
                            mi = u * 2 + t
                            h5 = hsp.tile([128, 1024], bf16, name="h5",
                                          tag="h5")
                            nc.scalar.activation(h5[:], mg[t][:], COPY,
                                                 scale=float(SCALE))
                            ia = mk.tile([128, 1024], bf16, name="ia",
                                         tag="ia")
                            nc.scalar.activation(
                                ia[:], mg[t][:], SIG, scale=float(SIGBIG),
                                bias=sgbias[:, rb:rb + 1])
                            masked_tile(mg[t], h5, ia, ob, t, mi)
                        g0 = u * 2048
                        nc.sync.dma_start(out[r0:r0 + 128, g0:g0 + 2048],
                                          ob[:])

    nc.compile()
    return nc


_nc_cache = None


def get_nc():
    global _nc_cache
    if _nc_cache is None:
        _nc_cache = build_nc()
    return _nc_cache


def kernel_with_result(x, trace: bool = False):
    x = np.ascontiguousarray(np.asarray(x), dtype=np.float32)
    assert x.shape == (N, DIM)
    nc = get_nc()
    xT = np.ascontiguousarray(x.T)
    in_maps = []
    for i in range(NCORES):
        xg = np.ascontiguousarray(x[i * RPC:(i + 1) * RPC, :].T)
        in_maps.append({"xT": xT, "xgT": xg})
    res = run_bass_kernel_spmd(nc, in_maps, core_ids=list(range(NCORES)),
                               trace=trace)
    outp = np.concatenate(
        [np.asarray(res.results[i]["out"]).astype(np.float32)
         for i in range(NCORES)], axis=0)
    return outp, res


def kernel(x) -> np.ndarray:
    outp, _res = kernel_with_result(x)
    return outp
